# revision 14
# baseline (speedup 1.0000x reference)
"""Bass/Tile TRN2 kernel for nn_Attention_26388279067013.

Computes, for each batch row b:
    feat = enc @ We.T + dec @ Ws.T + cov[:,None] * Wc.sum(1) + b     [S, H]
    att  = tanh(feat) @ v_w                                          [S]
    att[s >= L_b] = -inf ; w = softmax(att) ; new_cov = cov + w
Returns (attention_weights [B,S], new_coverage [B,S]) both float32.

Sharding: data-parallel over B across 8 NeuronCores (4 rows each).
"""

import sys

sys.path.insert(0, "/opt/trn_rl_repo")

import numpy as np

import concourse.bacc as bacc
import concourse.tile as tile
import concourse.mybir as mybir
from concourse.bass_utils import run_bass_kernel_spmd

B, S, H, D = 32, 4096, 512, 256
N_CORES = 8
B_LOC = B // N_CORES          # 4 batch rows per core
F32 = mybir.dt.float32
F32R = mybir.dt.float32r
ALU = mybir.AluOpType
ACTF = mybir.ActivationFunctionType

N_K = H // 128                # 4 contraction tiles
N_STILE = S // 128            # 32 psum s-tiles per batch
N_CHUNK = S // 512            # 8 dma chunks per batch
NEG_BIG = -30000.0            # exp(x - 30000) == 0.0 exactly in f32


def r32(ap):
    return ap.bitcast(F32R)


def build_kernel():
    nc = bacc.Bacc("TRN2", debug=False, num_devices=N_CORES)

    # ---- dram I/O (per core) ----
    encT = nc.dram_tensor("encT", [B_LOC, H, S], F32, kind="ExternalInput").ap()
    cov = nc.dram_tensor("cov", [B_LOC, 32, 128], F32, kind="ExternalInput").ap()
    dec_cols = nc.dram_tensor("dec_cols", [B_LOC, 128, 2], F32, kind="ExternalInput").ap()
    lens = nc.dram_tensor("lens", [B_LOC, 1], F32, kind="ExternalInput").ap()
    WeT = nc.dram_tensor("WeT", [H, H], F32, kind="ExternalInput").ap()
    WcT = nc.dram_tensor("WcT", [H, H], F32, kind="ExternalInput").ap()
    WsT = nc.dram_tensor("WsT", [D, H], F32, kind="ExternalInput").ap()
    b_row = nc.dram_tensor("b_row", [1, H], F32, kind="ExternalInput").ap()
    v_row = nc.dram_tensor("v_row", [1, H], F32, kind="ExternalInput").ap()
    iota_d = nc.dram_tensor("iota_pm", [128, 32], F32, kind="ExternalInput").ap()
    ident_d = nc.dram_tensor("ident", [128, 128], F32, kind="ExternalInput").ap()
    ones_row = nc.dram_tensor("ones_row", [1, S], F32, kind="ExternalInput").ap()
    out_w = nc.dram_tensor("out_w", [B_LOC, 32, 128], F32, kind="ExternalOutput").ap()
    out_c = nc.dram_tensor("out_c", [B_LOC, 32, 128], F32, kind="ExternalOutput").ap()

    with tile.TileContext(nc) as tc:
        with (
            tc.tile_pool(name="persist", bufs=1) as pp,
            tc.tile_pool(name="enc", bufs=12) as encp,
            tc.tile_pool(name="x", bufs=3) as xp,
            tc.tile_pool(name="scratch", bufs=2) as scrp,
            tc.tile_pool(name="small", bufs=4) as smp,
            tc.tile_pool(name="batch", bufs=3) as bp,
            tc.tile_pool(name="psum", bufs=3, space="PSUM") as psp,
            tc.tile_pool(name="psum_misc", bufs=2, space="PSUM") as psm,
        ):
            # ---- one-time setup ----
            wet = []
            for k in range(N_K):
                t = pp.tile([128, H], F32R, tag=f"wet{k}")
                nc.scalar.dma_start(t[:], r32(WeT[k * 128:(k + 1) * 128, :]))
                wet.append(t)
            wst = []
            for k in range(D // 128):
                t = pp.tile([128, H], F32, tag=f"wst{k}")
                nc.scalar.dma_start(t[:], WsT[k * 128:(k + 1) * 128, :])
                wst.append(t)
            brow_sb = pp.tile([1, H], F32, tag="brow")
            nc.scalar.dma_start(brow_sb[:], b_row[:, :])
            vrow_sb = pp.tile([1, H], F32, tag="vrow")
            nc.scalar.dma_start(vrow_sb[:], v_row[:, :])
            ones_k1 = pp.tile([1, 128], F32, tag="ones_k1")
            nc.vector.memset(ones_k1[:], 1.0)
            ones_col = pp.tile([128, 1], F32, tag="ones_col")
            nc.vector.memset(ones_col[:], 1.0)

            # dep-free matmul burst: trips the PE HAM to K=8/8 (~2.4 GHz)
            # before the real stream arrives, instead of ~40us into it.
            warm_f = pp.tile([128, 512], F32, tag="warm_f")
            nc.vector.memset(warm_f[:], 0.5)
            warm = pp.tile([128, 512], F32R, tag="warm")
            nc.scalar.dma_start(warm[:], r32(warm_f[:]))
            for wi in range(32):
                ps_w = psm.tile([128, 512], F32, tag="mpsum")
                nc.tensor.matmul(ps_w[:], warm[:, 0:128], warm[:],
                                 start=True, stop=True)

            # wc_sum[o] = sum_h WcT[h, o]  -> [1, 512]
            ps_wc = psm.tile([1, H], F32, tag="mpsum")
            for k in range(N_K):
                t = scrp.tile([128, H], F32, tag="wct")
                nc.scalar.dma_start(t[:], WcT[k * 128:(k + 1) * 128, :])
                nc.tensor.matmul(ps_wc[:], ones_col[:], t[:],
                                 start=(k == 0), stop=(k == N_K - 1))
            wc_row = pp.tile([1, H], F32, tag="wc_row")
            nc.scalar.copy(wc_row[:], ps_wc[:])

            # v_bcast[p, o] = v_w[o]
            ps_vb = psm.tile([128, H], F32, tag="mpsum")
            nc.tensor.matmul(ps_vb[:], ones_k1[:], vrow_sb[:],
                             start=True, stop=True)
            v_bcast = pp.tile([128, H], F32, tag="v_bcast")
            nc.scalar.copy(v_bcast[:], ps_vb[:])

            iota_sb = pp.tile([128, 32], F32, tag="iota")
            ident_sb = pp.tile([128, 128], F32, tag="ident")

            # ---- per batch, software-pipelined ----
            # prep(b) builds per-batch small operands; heavy(b) is the matmul
            # stream; softmax(b) is emitted in the middle of heavy(b+1) so the
            # PE never drains at a batch boundary (keeps HAM warm).
            state = {}

            def emit_prep(b):
                dc = smp.tile([128, 2], F32, tag="dc")
                nc.scalar.dma_start(dc[:], dec_cols[b, :, :])
                ps_row = psm.tile([1, H], F32, tag="mpsum")
                for j in range(D // 128):
                    nc.tensor.matmul(ps_row[:], dc[:, j:j + 1], wst[j][:],
                                     start=(j == 0), stop=(j == 1))
                aug_st = bp.tile([2, H], F32, tag="aug_st")
                nc.vector.tensor_tensor(aug_st[0:1, :], ps_row[:], brow_sb[:], ALU.add)
                nc.scalar.dma_start(aug_st[1:2, :], wc_row[:])
                aug_rhs = bp.tile([2, H], F32R, tag="aug_rhs")
                nc.scalar.dma_start(aug_rhs[:], r32(aug_st[:]))

                cov_aug = bp.tile([2, S], F32R, tag="cov_aug")
                nc.scalar.dma_start(cov_aug[0:1, :], r32(ones_row[:, :]))
                nc.scalar.dma_start(
                    cov_aug[1:2, :],
                    r32(cov[b:b + 1].rearrange("c a b -> c (a b)")),
                )

                len_sb = smp.tile([1, 1], F32, tag="len_sb")
                nc.scalar.dma_start(len_sb[:], lens[b:b + 1, :])
                ps_l = psm.tile([128, 1], F32, tag="mpsum")
                nc.tensor.matmul(ps_l[:], ones_k1[:], len_sb[:],
                                 start=True, stop=True)
                l_col = smp.tile([128, 1], F32, tag="l_col")
                nc.scalar.copy(l_col[:], ps_l[:])

                att_pm = bp.tile([128, 32], F32, tag="att_pm")
                state[b] = dict(aug_rhs=aug_rhs, cov_aug=cov_aug,
                                l_col=l_col, att_pm=att_pm)

            def emit_heavy_chunk(b, c):
                st8 = state[b]
                ek = []
                for k in range(N_K):
                    t = encp.tile([128, 512], F32R, tag="enc")
                    nc.sync.dma_start(
                        t[:], r32(encT[b, k * 128:(k + 1) * 128, c * 512:(c + 1) * 512]))
                    ek.append(t)
                for t2 in range(2):
                    ps = psp.tile([128, 1024], F32, tag="feat")
                    for half in range(2):
                        st = 4 * c + 2 * t2 + half
                        scol = (2 * t2 + half) * 128
                        dst = ps[:, half * 512:(half + 1) * 512]
                        for k in range(N_K):
                            nc.tensor.matmul(
                                dst, ek[k][:, scol:scol + 128], wet[k][:],
                                start=(k == 0), stop=False)
                        nc.tensor.matmul(
                            dst, st8["cov_aug"][:, st * 128:(st + 1) * 128],
                            st8["aug_rhs"][:], start=False, stop=True)
                    x = xp.tile([128, 1024], F32, tag="x")
                    nc.scalar.activation(x[:], ps[:], ACTF.Tanh)
                    for half in range(2):
                        st = 4 * c + 2 * t2 + half
                        scr = scrp.tile([128, 512], F32, tag="vscr")
                        nc.vector.scalar_tensor_tensor(
                            scr[:], x[:, half * 512:(half + 1) * 512],
                            1.0, v_bcast[:], ALU.bypass, ALU.mult,
                            accum_out=st8["att_pm"][:, st:st + 1])

            def emit_softmax(b):
                st8 = state.pop(b)
                att_pm, l_col = st8["att_pm"], st8["l_col"]
                pad01 = bp.tile([128, 32], F32, tag="pad01")
                nc.vector.tensor_scalar(pad01[:], iota_sb[:], l_col[:], None, ALU.is_ge)
                att_m = bp.tile([128, 32], F32, tag="att_m")
                nc.vector.scalar_tensor_tensor(
                    att_m[:], pad01[:], NEG_BIG, att_pm[:], ALU.mult, ALU.add)
                exp_pm = bp.tile([128, 32], F32, tag="exp_pm")
                rowsum = smp.tile([128, 1], F32, tag="rowsum")
                nc.scalar.activation(exp_pm[:], att_m[:], ACTF.Exp, accum_out=rowsum[:])
                ps_d = psm.tile([1, 1], F32, tag="mpsum")
                nc.tensor.matmul(ps_d[:], rowsum[:], ones_col[:],
                                 start=True, stop=True)
                rinv = smp.tile([1, 1], F32, tag="rinv")
                nc.vector.reciprocal(rinv[:], ps_d[:])
                ps_r = psm.tile([128, 1], F32, tag="mpsum")
                nc.tensor.matmul(ps_r[:], ones_k1[:], rinv[:],
                                 start=True, stop=True)
                rinv_col = smp.tile([128, 1], F32, tag="rinv_col")
                nc.scalar.copy(rinv_col[:], ps_r[:])
                w_pm = bp.tile([128, 32], F32, tag="w_pm")
                nc.vector.tensor_scalar(w_pm[:], exp_pm[:], rinv_col[:], None, ALU.mult)

                ps_t = psm.tile([32, 128], F32, tag="mpsum")
                nc.tensor.transpose(ps_t[:], w_pm[:], ident_sb[:])
                covT = bp.tile([32, 128], F32, tag="covT")
                nc.scalar.dma_start(covT[:], cov[b, :, :])
                w_sb = bp.tile([32, 128], F32, tag="w_sb")
                nc.scalar.copy(w_sb[:], ps_t[:])
                ncov = bp.tile([32, 128], F32, tag="ncov")
                nc.vector.tensor_tensor(ncov[:], ps_t[:], covT[:], ALU.add)
                nc.scalar.dma_start(out_w[b, :, :], w_sb[:])
                nc.scalar.dma_start(out_c[b, :, :], ncov[:])

            emit_prep(0)
            emit_prep(1)
            nc.scalar.dma_start(iota_sb[:], iota_d[:, :])
            nc.scalar.dma_start(ident_sb[:], ident_d[:, :])
            for b in range(B_LOC):
                for c in range(N_CHUNK):
                    emit_heavy_chunk(b, c)
                    if c == 2 and b >= 1:
                        emit_softmax(b - 1)
                    if c == 5 and b + 2 < B_LOC:
                        emit_prep(b + 2)
            emit_softmax(B_LOC - 1)

    nc.compile()
    return nc


_NC_CACHE = {}


def _get_nc():
    if "nc" not in _NC_CACHE:
        _NC_CACHE["nc"] = build_kernel()
    return _NC_CACHE["nc"]


def make_in_maps(dec_input, enc_output, coverage_vector, text_lengths, W, b, v_w, v_b):
    dec_input = np.asarray(dec_input, np.float32)
    enc_output = np.ascontiguousarray(np.asarray(enc_output, np.float32))
    coverage_vector = np.asarray(coverage_vector, np.float32)
    lens_f = np.asarray(text_lengths).astype(np.float32)
    W = np.asarray(W, np.float32)
    b = np.asarray(b, np.float32)
    v_w = np.asarray(v_w, np.float32)

    WeT = np.ascontiguousarray(W[:, :H].T)            # [H, H]
    WsT = np.ascontiguousarray(W[:, H:H + D].T)       # [D, H]
    WcT = np.ascontiguousarray(W[:, H + D:].T)        # [H, H]
    b_rw = np.ascontiguousarray(b[None, :])
    v_rw = np.ascontiguousarray(v_w[None, :])
    iota_pm = (np.arange(32)[None, :] * 128 + np.arange(128)[:, None]).astype(np.float32)
    ident = np.eye(128, dtype=np.float32)

    in_maps = []
    for core in range(N_CORES):
        lo = core * B_LOC
        hi = lo + B_LOC
        encT = np.ascontiguousarray(enc_output[lo:hi].transpose(0, 2, 1))  # [B_LOC, H, S]
        covc = np.ascontiguousarray(coverage_vector[lo:hi].reshape(B_LOC, 32, 128))
        decc = np.ascontiguousarray(
            dec_input[lo:hi, 0, :].reshape(B_LOC, 2, 128).transpose(0, 2, 1))
        in_maps.append({
            "encT": encT,
            "cov": covc,
            "dec_cols": decc,
            "lens": np.ascontiguousarray(lens_f[lo:hi].reshape(B_LOC, 1)),
            "WeT": WeT, "WcT": WcT, "WsT": WsT,
            "b_row": b_rw, "v_row": v_rw,
            "iota_pm": iota_pm, "ident": ident,
            "ones_row": np.ones((1, S), np.float32),
        })
    return in_maps


def kernel(dec_input, enc_output, coverage_vector, text_lengths, W, b, v_w, v_b,
           _trace=False):
    nc = _get_nc()
    in_maps = make_in_maps(dec_input, enc_output, coverage_vector, text_lengths,
                           W, b, v_w, v_b)
    res = run_bass_kernel_spmd(nc, in_maps, list(range(N_CORES)), trace=_trace)
    w = np.concatenate([r["out_w"].reshape(B_LOC, S) for r in res.results], axis=0)
    c = np.concatenate([r["out_c"].reshape(B_LOC, S) for r in res.results], axis=0)
    if _trace:
        kernel.last_result = res
    return w, c


# revision 16
# speedup vs baseline: 1.0128x; 1.0128x over previous
"""Bass/Tile TRN2 kernel for nn_Attention_26388279067013.

Computes, for each batch row b:
    feat = enc @ We.T + dec @ Ws.T + cov[:,None] * Wc.sum(1) + b     [S, H]
    att  = tanh(feat) @ v_w                                          [S]
    att[s >= L_b] = -inf ; w = softmax(att) ; new_cov = cov + w
Returns (attention_weights [B,S], new_coverage [B,S]) both float32.

Sharding: data-parallel over B across 8 NeuronCores (4 rows each).
"""

import sys

sys.path.insert(0, "/opt/trn_rl_repo")

import numpy as np

import concourse.bacc as bacc
import concourse.tile as tile
import concourse.mybir as mybir
from concourse.bass_utils import run_bass_kernel_spmd

B, S, H, D = 32, 4096, 512, 256
N_CORES = 8
B_LOC = B // N_CORES          # 4 batch rows per core
F32 = mybir.dt.float32
F32R = mybir.dt.float32r
ALU = mybir.AluOpType
ACTF = mybir.ActivationFunctionType

N_K = H // 128                # 4 contraction tiles
N_STILE = S // 128            # 32 psum s-tiles per batch
N_CHUNK = S // 512            # 8 dma chunks per batch
NEG_BIG = -30000.0            # exp(x - 30000) == 0.0 exactly in f32


def r32(ap):
    return ap.bitcast(F32R)


def build_kernel():
    nc = bacc.Bacc("TRN2", debug=False, num_devices=N_CORES)

    # ---- dram I/O (per core) ----
    encT = nc.dram_tensor("encT", [B_LOC, H, S], F32, kind="ExternalInput").ap()
    cov = nc.dram_tensor("cov", [B_LOC, 32, 128], F32, kind="ExternalInput").ap()
    dec_cols = nc.dram_tensor("dec_cols", [B_LOC, 128, 2], F32, kind="ExternalInput").ap()
    lens = nc.dram_tensor("lens", [B_LOC, 1], F32, kind="ExternalInput").ap()
    WeT = nc.dram_tensor("WeT", [H, H], F32, kind="ExternalInput").ap()
    WcT = nc.dram_tensor("WcT", [H, H], F32, kind="ExternalInput").ap()
    WsT = nc.dram_tensor("WsT", [D, H], F32, kind="ExternalInput").ap()
    b_row = nc.dram_tensor("b_row", [1, H], F32, kind="ExternalInput").ap()
    v_row = nc.dram_tensor("v_row", [1, H], F32, kind="ExternalInput").ap()
    iota_d = nc.dram_tensor("iota_pm", [128, 32], F32, kind="ExternalInput").ap()
    ident_d = nc.dram_tensor("ident", [128, 128], F32, kind="ExternalInput").ap()
    ones_row = nc.dram_tensor("ones_row", [1, S], F32, kind="ExternalInput").ap()
    out_w = nc.dram_tensor("out_w", [B_LOC, 32, 128], F32, kind="ExternalOutput").ap()
    out_c = nc.dram_tensor("out_c", [B_LOC, 32, 128], F32, kind="ExternalOutput").ap()

    with tile.TileContext(nc) as tc:
        with (
            tc.tile_pool(name="persist", bufs=1) as pp,
            tc.tile_pool(name="enc", bufs=12) as encp,
            tc.tile_pool(name="x", bufs=3) as xp,
            tc.tile_pool(name="scratch", bufs=2) as scrp,
            tc.tile_pool(name="small", bufs=4) as smp,
            tc.tile_pool(name="batch", bufs=3) as bp,
            tc.tile_pool(name="psum", bufs=4, space="PSUM") as psp,
            tc.tile_pool(name="psum_misc", bufs=4, space="PSUM") as psm,
        ):
            # ---- one-time setup ----
            wet = []
            for k in range(N_K):
                t = pp.tile([128, H], F32R, tag=f"wet{k}")
                nc.scalar.dma_start(t[:], r32(WeT[k * 128:(k + 1) * 128, :]))
                wet.append(t)
            wst = []
            for k in range(D // 128):
                t = pp.tile([128, H], F32, tag=f"wst{k}")
                nc.scalar.dma_start(t[:], WsT[k * 128:(k + 1) * 128, :])
                wst.append(t)
            brow_sb = pp.tile([1, H], F32, tag="brow")
            nc.scalar.dma_start(brow_sb[:], b_row[:, :])
            vrow_sb = pp.tile([1, H], F32, tag="vrow")
            nc.scalar.dma_start(vrow_sb[:], v_row[:, :])
            ones_k1 = pp.tile([1, 128], F32, tag="ones_k1")
            nc.vector.memset(ones_k1[:], 1.0)
            ones_col = pp.tile([128, 1], F32, tag="ones_col")
            nc.vector.memset(ones_col[:], 1.0)

            # dep-free matmul burst: trips the PE HAM to K=8/8 (~2.4 GHz)
            # before the real stream arrives, instead of ~40us into it.
            warm_f = pp.tile([128, 512], F32, tag="warm_f")
            nc.vector.memset(warm_f[:], 0.5)
            warm = pp.tile([128, 512], F32R, tag="warm")
            nc.scalar.dma_start(warm[:], r32(warm_f[:]))
            for wi in range(20):
                ps_w = psm.tile([128, 512], F32, tag="mpsum")
                nc.tensor.matmul(ps_w[:], warm[:, 0:128], warm[:],
                                 start=True, stop=True)

            # wc_sum[o] = sum_h WcT[h, o]  -> [1, 512]
            ps_wc = psm.tile([1, H], F32, tag="mpsum")
            for k in range(N_K):
                t = scrp.tile([128, H], F32, tag="wct")
                nc.scalar.dma_start(t[:], WcT[k * 128:(k + 1) * 128, :])
                nc.tensor.matmul(ps_wc[:], ones_col[:], t[:],
                                 start=(k == 0), stop=(k == N_K - 1))
            wc_row = pp.tile([1, H], F32, tag="wc_row")
            nc.scalar.copy(wc_row[:], ps_wc[:])

            # v_bcast[p, o] = v_w[o]
            ps_vb = psm.tile([128, H], F32, tag="mpsum")
            nc.tensor.matmul(ps_vb[:], ones_k1[:], vrow_sb[:],
                             start=True, stop=True)
            v_bcast = pp.tile([128, H], F32, tag="v_bcast")
            nc.scalar.copy(v_bcast[:], ps_vb[:])

            iota_sb = pp.tile([128, 32], F32, tag="iota")
            ident_sb = pp.tile([128, 128], F32, tag="ident")

            # ---- per batch, software-pipelined ----
            # prep(b) builds per-batch small operands; heavy(b) is the matmul
            # stream; softmax(b) is emitted in the middle of heavy(b+1) so the
            # PE never drains at a batch boundary (keeps HAM warm).
            state = {}

            def emit_prep(b):
                dc = smp.tile([128, 2], F32, tag="dc")
                nc.scalar.dma_start(dc[:], dec_cols[b, :, :])
                ps_row = psm.tile([1, H], F32, tag="mpsum")
                for j in range(D // 128):
                    nc.tensor.matmul(ps_row[:], dc[:, j:j + 1], wst[j][:],
                                     start=(j == 0), stop=(j == 1))
                aug_st = bp.tile([2, H], F32, tag="aug_st")
                nc.vector.tensor_tensor(aug_st[0:1, :], ps_row[:], brow_sb[:], ALU.add)
                nc.scalar.dma_start(aug_st[1:2, :], wc_row[:])
                aug_rhs = bp.tile([2, H], F32R, tag="aug_rhs")
                nc.scalar.dma_start(aug_rhs[:], r32(aug_st[:]))

                cov_aug = bp.tile([2, S], F32R, tag="cov_aug")
                nc.scalar.dma_start(cov_aug[0:1, :], r32(ones_row[:, :]))
                nc.scalar.dma_start(
                    cov_aug[1:2, :],
                    r32(cov[b:b + 1].rearrange("c a b -> c (a b)")),
                )

                len_sb = smp.tile([1, 1], F32, tag="len_sb")
                nc.scalar.dma_start(len_sb[:], lens[b:b + 1, :])
                ps_l = psm.tile([128, 1], F32, tag="mpsum")
                nc.tensor.matmul(ps_l[:], ones_k1[:], len_sb[:],
                                 start=True, stop=True)
                l_col = smp.tile([128, 1], F32, tag="l_col")
                nc.scalar.copy(l_col[:], ps_l[:])

                att_pm = bp.tile([128, 32], F32, tag="att_pm")
                state[b] = dict(aug_rhs=aug_rhs, cov_aug=cov_aug,
                                l_col=l_col, att_pm=att_pm)

            def emit_heavy_chunk(b, c):
                st8 = state[b]
                ek = []
                for k in range(N_K):
                    t = encp.tile([128, 512], F32R, tag="enc")
                    nc.sync.dma_start(
                        t[:], r32(encT[b, k * 128:(k + 1) * 128, c * 512:(c + 1) * 512]))
                    ek.append(t)
                for q in range(4):
                    st = 4 * c + q
                    scol = q * 128
                    ps = psp.tile([128, 512], F32, tag="feat")
                    for k in range(N_K):
                        nc.tensor.matmul(
                            ps[:], ek[k][:, scol:scol + 128], wet[k][:],
                            start=(k == 0), stop=False)
                    nc.tensor.matmul(
                        ps[:], st8["cov_aug"][:, st * 128:(st + 1) * 128],
                        st8["aug_rhs"][:], start=False, stop=True)
                    x = xp.tile([128, 512], F32, tag="x")
                    nc.scalar.activation(x[:], ps[:], ACTF.Tanh)
                    scr = scrp.tile([128, 512], F32, tag="vscr")
                    nc.vector.scalar_tensor_tensor(
                        scr[:], x[:], 1.0, v_bcast[:], ALU.bypass, ALU.mult,
                        accum_out=st8["att_pm"][:, st:st + 1])

            def emit_softmax(b):
                st8 = state.pop(b)
                att_pm, l_col = st8["att_pm"], st8["l_col"]
                pad01 = bp.tile([128, 32], F32, tag="pad01")
                nc.vector.tensor_scalar(pad01[:], iota_sb[:], l_col[:], None, ALU.is_ge)
                att_m = bp.tile([128, 32], F32, tag="att_m")
                nc.vector.scalar_tensor_tensor(
                    att_m[:], pad01[:], NEG_BIG, att_pm[:], ALU.mult, ALU.add)
                exp_pm = bp.tile([128, 32], F32, tag="exp_pm")
                rowsum = smp.tile([128, 1], F32, tag="rowsum")
                nc.scalar.activation(exp_pm[:], att_m[:], ACTF.Exp, accum_out=rowsum[:])
                ps_d = psm.tile([1, 1], F32, tag="mpsum")
                nc.tensor.matmul(ps_d[:], rowsum[:], ones_col[:],
                                 start=True, stop=True)
                rinv = smp.tile([1, 1], F32, tag="rinv")
                nc.vector.reciprocal(rinv[:], ps_d[:])
                ps_r = psm.tile([128, 1], F32, tag="mpsum")
                nc.tensor.matmul(ps_r[:], ones_k1[:], rinv[:],
                                 start=True, stop=True)
                rinv_col = smp.tile([128, 1], F32, tag="rinv_col")
                nc.scalar.copy(rinv_col[:], ps_r[:])
                w_pm = bp.tile([128, 32], F32, tag="w_pm")
                nc.vector.tensor_scalar(w_pm[:], exp_pm[:], rinv_col[:], None, ALU.mult)

                ps_t = psm.tile([32, 128], F32, tag="mpsum")
                nc.tensor.transpose(ps_t[:], w_pm[:], ident_sb[:])
                covT = bp.tile([32, 128], F32, tag="covT")
                nc.scalar.dma_start(covT[:], cov[b, :, :])
                w_sb = bp.tile([32, 128], F32, tag="w_sb")
                nc.scalar.copy(w_sb[:], ps_t[:])
                ncov = bp.tile([32, 128], F32, tag="ncov")
                nc.vector.tensor_tensor(ncov[:], ps_t[:], covT[:], ALU.add)
                nc.scalar.dma_start(out_w[b, :, :], w_sb[:])
                nc.scalar.dma_start(out_c[b, :, :], ncov[:])

            emit_prep(0)
            emit_prep(1)
            nc.scalar.dma_start(iota_sb[:], iota_d[:, :])
            nc.scalar.dma_start(ident_sb[:], ident_d[:, :])
            for b in range(B_LOC):
                for c in range(N_CHUNK):
                    emit_heavy_chunk(b, c)
                    if c == 2 and b >= 1:
                        emit_softmax(b - 1)
                    if c == 5 and b + 2 < B_LOC:
                        emit_prep(b + 2)
            emit_softmax(B_LOC - 1)

    nc.compile()
    return nc


_NC_CACHE = {}


def _get_nc():
    if "nc" not in _NC_CACHE:
        _NC_CACHE["nc"] = build_kernel()
    return _NC_CACHE["nc"]


def make_in_maps(dec_input, enc_output, coverage_vector, text_lengths, W, b, v_w, v_b):
    dec_input = np.asarray(dec_input, np.float32)
    enc_output = np.ascontiguousarray(np.asarray(enc_output, np.float32))
    coverage_vector = np.asarray(coverage_vector, np.float32)
    lens_f = np.asarray(text_lengths).astype(np.float32)
    W = np.asarray(W, np.float32)
    b = np.asarray(b, np.float32)
    v_w = np.asarray(v_w, np.float32)

    WeT = np.ascontiguousarray(W[:, :H].T)            # [H, H]
    WsT = np.ascontiguousarray(W[:, H:H + D].T)       # [D, H]
    WcT = np.ascontiguousarray(W[:, H + D:].T)        # [H, H]
    b_rw = np.ascontiguousarray(b[None, :])
    v_rw = np.ascontiguousarray(v_w[None, :])
    iota_pm = (np.arange(32)[None, :] * 128 + np.arange(128)[:, None]).astype(np.float32)
    ident = np.eye(128, dtype=np.float32)

    in_maps = []
    for core in range(N_CORES):
        lo = core * B_LOC
        hi = lo + B_LOC
        encT = np.ascontiguousarray(enc_output[lo:hi].transpose(0, 2, 1))  # [B_LOC, H, S]
        covc = np.ascontiguousarray(coverage_vector[lo:hi].reshape(B_LOC, 32, 128))
        decc = np.ascontiguousarray(
            dec_input[lo:hi, 0, :].reshape(B_LOC, 2, 128).transpose(0, 2, 1))
        in_maps.append({
            "encT": encT,
            "cov": covc,
            "dec_cols": decc,
            "lens": np.ascontiguousarray(lens_f[lo:hi].reshape(B_LOC, 1)),
            "WeT": WeT, "WcT": WcT, "WsT": WsT,
            "b_row": b_rw, "v_row": v_rw,
            "iota_pm": iota_pm, "ident": ident,
            "ones_row": np.ones((1, S), np.float32),
        })
    return in_maps


def kernel(dec_input, enc_output, coverage_vector, text_lengths, W, b, v_w, v_b,
           _trace=False):
    nc = _get_nc()
    in_maps = make_in_maps(dec_input, enc_output, coverage_vector, text_lengths,
                           W, b, v_w, v_b)
    res = run_bass_kernel_spmd(nc, in_maps, list(range(N_CORES)), trace=_trace)
    w = np.concatenate([r["out_w"].reshape(B_LOC, S) for r in res.results], axis=0)
    c = np.concatenate([r["out_c"].reshape(B_LOC, S) for r in res.results], axis=0)
    if _trace:
        kernel.last_result = res
    return w, c
